# revision 18
# baseline (speedup 1.0000x reference)
"""Trainium2 Bass kernel for nn_HT_56298431316042 (histogram_binning).

Computes  out = relu(image.reshape(32, 16384)) @ vote.reshape(16384, 16384) / 128
         -> reshape (2, 16, 128, 128)

Sharding: column-wise over the 16384 Hough bins -> 2048 bins per core, 8 cores,
no communication.

The vote matrix is binary, so 1 byte/entry (fp8) wastes 7 bits. This kernel
ships most of V bit-packed at 4 entries/byte and expands on-device:

  - Host packs V[k, j] into uint16 words: word (p, g, q, w) carries, at bit
    i the entry for k-chunk (8g+4q+i) / bin 2w, and at bit (8+i) the same
    chunk / bin 2w+1.  (k-chunk = 128 consecutive k rows; p = k % 128.)
  - DVE tensor_scalar (w >> i) & 0x0101 (one dual-op instruction per
    bit-class, uint16 4x perf mode) extracts one byte per bin whose bit
    pattern is 0x01: as fp8e4m3 that is exactly 2^-9, so every expanded
    plane equals V * 2^-9 elementwise.
  - x is pre-scaled on host by 2^9 * 2^-7 = 4 (the 2^-7 is the reference's
    /128), so psum accumulates the final output directly; fp8 quantization
    of relu(x*4) is the only rounding (rel err ~3.7e-3).
  - The expanded planes are the matmul's *stationary* operand and x (just
    32 wide) the moving one, with psum indexed by bin; HBM traffic for V
    drops 4x (33.6 -> 8.4 MB/core) and the DVE expansion (~33 us) is the
    pipeline spine.
  - Two superblocks ride as pre-expanded fp8 planes straight from HBM
    (byte 0x01 = same 2^-9 value), trading idle DMA bandwidth for DVE time
    to balance the two spines.

Engine budget per core (cost model): DVE expand ~33 us / DMA ~34 us
(balanced spines), PE ~9 us, ACT only x-relu + epilogue staging.  V DMAs
issue from the SP queue so descriptor generation never stalls the ACT-side
x prep; x ships in two chunks so the first matmuls start ~5 us in.
"""

import numpy as np

import concourse.bass as bass
import concourse.bacc as bacc
import concourse.mybir as mybir
import concourse.tile as tile
from concourse.bass_utils import run_bass_kernel_spmd

NCORES = 8
B, C, ROWS, COLS, H, W = 2, 16, 128, 128, 128, 128
BC = B * C                      # 32 output rows
K = ROWS * COLS                 # 16384 contraction
NTOT = H * W                    # 16384 output bins
NPC = NTOT // NCORES            # 2048 bins per core
KC = K // 128                   # 128 k-chunks of 128 rows
NSB = 16                        # superblocks of 8 k-chunks
CPS = KC // NSB                 # 8 chunks per superblock
PPS = 2 * CPS                   # planes (chunks) per 2-SB slab = 16
NG = NPC // 128                 # psum bin groups
RSLAB = 3                       # expanded-ring depth (slabs)
XCH0 = 32                       # chunks covered by the first x chunk

# packed superblocks, in processing order; SBs 12,13 ship dense
PACKED_SBS = (0, 1, 2, 3, 4, 5, 6, 7, 8, 9, 10, 11, 14, 15)
NPK = len(PACKED_SBS)
DENSE_BASE = 96                 # first chunk of the dense slab (SB 12)

# plane byte 0x01 = e4m3 2^-9; x carries 2^9 * 2^-7 (the /128) = 4.0
X_SCALE = 4.0

_nc_cache: dict[str, bass.Bass] = {}


def _build() -> bass.Bass:
    if "bp4" in _nc_cache:
        return _nc_cache["bp4"]
    f32 = mybir.dt.float32
    f8 = mybir.dt.float8e4
    u16 = mybir.dt.uint16

    nc = bacc.Bacc("TRN2", target_bir_lowering=False, debug=False,
                   num_devices=NCORES)
    x_dram = nc.dram_tensor("x", (128, KC * BC), f8, kind="ExternalInput")
    v_dram = nc.dram_tensor("v", (NPK, 128, NPC), u16, kind="ExternalInput")
    d_dram = nc.dram_tensor("vd", (128, PPS * NPC), f8, kind="ExternalInput")
    o_dram = nc.dram_tensor("out", (128, NG * BC), f32, kind="ExternalOutput")

    with tile.TileContext(nc) as tc:
        with tc.tile_pool(name="xp", bufs=1) as xp, \
             tc.tile_pool(name="vp", bufs=1) as vp, \
             tc.tile_pool(name="sp", bufs=1) as sp, \
             tc.tile_pool(name="dp", bufs=1) as dp, \
             tc.tile_pool(name="pp", bufs=1, space="PSUM") as pp, \
             tc.tile_pool(name="op", bufs=1) as op:

            x_raw = xp.tile([128, KC * BC], f8)
            packed = vp.tile([128, NPK * NPC], u16)
            ring = sp.tile([128, RSLAB * PPS * NPC], f8)
            dense = dp.tile([128, PPS * NPC], f8)
            x8 = xp.tile([128, KC * BC], f8)
            psum = pp.tile([128, NG * BC], f32)

            # DMA-engine arrival order: x chunk 0, vq0, vq1, x chunk 1,
            # vq2..vq13, dense slab, output.  (x head on the ACT queue, all
            # V + x tail on the SP queue so descriptor generation never
            # stalls the ACT-side relu chain.)
            nc.scalar.dma_start(out=x_raw[:, :XCH0 * BC],
                                in_=x_dram.ap()[:, :XCH0 * BC])
            nc.sync.dma_start(out=packed[:, 0 * NPC:1 * NPC],
                              in_=v_dram.ap()[0])
            nc.sync.dma_start(out=packed[:, 1 * NPC:2 * NPC],
                              in_=v_dram.ap()[1])
            nc.sync.dma_start(out=packed[:, 2 * NPC:3 * NPC],
                              in_=v_dram.ap()[2])
            nc.sync.dma_start(out=packed[:, 3 * NPC:4 * NPC],
                              in_=v_dram.ap()[3])
            nc.sync.dma_start(out=x_raw[:, XCH0 * BC:],
                              in_=x_dram.ap()[:, XCH0 * BC:])
            for g in range(4, NPK):
                nc.sync.dma_start(out=packed[:, g * NPC:(g + 1) * NPC],
                                  in_=v_dram.ap()[g])
            nc.sync.dma_start(out=dense[:], in_=d_dram.ap())

            relu = mybir.ActivationFunctionType.Relu
            nc.scalar.activation(x8[:, :XCH0 * BC], x_raw[:, :XCH0 * BC], relu)
            nc.scalar.activation(x8[:, XCH0 * BC:], x_raw[:, XCH0 * BC:], relu)

            DR = mybir.MatmulPerfMode.DoubleRow
            shr = mybir.AluOpType.logical_shift_right
            band = mybir.AluOpType.bitwise_and
            first = [True]

            def mms(chunk_base, view, jj, pgs):
                """MMs for DR pairs (classes 2jj, 2jj+1), planes cl*4+pg."""
                for pg in pgs:
                    c = chunk_base + 4 * pg + 2 * jj
                    xmov = x8[:, c * BC:(c + 2) * BC].rearrange(
                        "p (j m) -> p j m", j=2)
                    for ng in range(NG):
                        nc.tensor.matmul(
                            psum[:, ng * BC:(ng + 1) * BC],
                            lhsT=view[:, 2 * jj:2 * jj + 2, pg,
                                      ng * 128:(ng + 1) * 128],
                            rhs=xmov,
                            start=first[0], stop=(c == KC - 2),
                            perf_mode=DR)
                        first[0] = False

            def and_op(out_ap, pin, i):
                nc.vector.tensor_scalar(
                    out=out_ap, in0=pin, scalar1=i, scalar2=0x0101,
                    op0=shr, op1=band)

            # slot schedule: (ring slab, packed tiles, plane cols, chunks)
            # slot 0 is split into two 1-SB halves for an earlier start
            dense8 = dense[:].rearrange("p (cl pg n) -> p cl pg n",
                                        cl=4, pg=4)
            n2slots = NPK // 2 - 1          # 2-SB slots after the split
            for s in range(1 + n2slots):
                slab = s % RSLAB
                if s == 0:
                    # two half-slots: SB0 -> pg 0,1 ; SB1 -> pg 2,3
                    roff = slab * PPS * NPC
                    rslot = ring[:, roff:roff + PPS * NPC]
                    r16 = rslot.bitcast(u16).rearrange(
                        "p (cl pg w) -> p cl pg w", cl=4, pg=4)
                    r8 = rslot.rearrange("p (cl pg n) -> p cl pg n",
                                         cl=4, pg=4)
                    for h in range(2):
                        pin = packed[:, h * NPC:(h + 1) * NPC]
                        for i in (0, 1):
                            and_op(r16[:, i, 2 * h:2 * h + 2], pin, i)
                    mms(0, r8, 0, range(4))
                    for h in range(2):
                        pin = packed[:, h * NPC:(h + 1) * NPC]
                        for i in (2, 3):
                            and_op(r16[:, i, 2 * h:2 * h + 2], pin, i)
                    mms(0, r8, 1, range(4))
                else:
                    pk = (2 * s, 2 * s + 1)
                    chunk_base = PACKED_SBS[pk[0]] * CPS
                    roff = slab * PPS * NPC
                    rslot = ring[:, roff:roff + PPS * NPC]
                    r16 = rslot.bitcast(u16).rearrange(
                        "p (cl w) -> p cl w", cl=4)
                    r8 = rslot.rearrange("p (cl pg n) -> p cl pg n",
                                         cl=4, pg=4)
                    pin = packed[:, pk[0] * NPC:(pk[0] + 2) * NPC]
                    for i in (0, 1):
                        and_op(r16[:, i], pin, i)
                    mms(chunk_base, r8, 0, range(4))
                    for i in (2, 3):
                        and_op(r16[:, i], pin, i)
                    mms(chunk_base, r8, 1, range(4))
                if s == 4:
                    # dense slab's MMs (chunks 96..111); its DMA lands late
                    # but the PE has plenty of slack here
                    for jj in range(2):
                        mms(DENSE_BASE, dense8, jj, range(4))

            # psum already holds out (the /128 is folded into x); PSUM can't
            # feed DMA directly, so stage halves via DVE and ACT in parallel
            NF = NG * BC
            out_t = op.tile([128, NF], f32)
            HN = NF // 2
            nc.vector.tensor_copy(out=out_t[:, :HN], in_=psum[:, :HN])
            nc.scalar.copy(out_t[:, HN:], psum[:, HN:])
            nc.sync.dma_start(out=o_dram.ap()[:, :HN], in_=out_t[:, :HN])
            nc.scalar.dma_start(out=o_dram.ap()[:, HN:], in_=out_t[:, HN:])

    nc.finalize()
    _nc_cache["bp4"] = nc
    return nc


def _prep_inputs(image: np.ndarray, vote_index: np.ndarray):
    f8np = mybir.dt.np(mybir.dt.float8e4)

    # x arranged (128, c*32+m) = image[m, c*128+p] * 4, signed fp8
    x2 = np.ascontiguousarray(image.reshape(BC, K), dtype=np.float32)
    x3 = x2.reshape(BC, KC, 128) * X_SCALE
    x_arr = np.ascontiguousarray(
        x3.transpose(2, 1, 0)).reshape(128, KC * BC).astype(f8np)

    Vb = (vote_index.reshape(K, NTOT) != 0)
    # packed V: word[g, p, q*1024+w] bits (i | 8+i) =
    #   V[(8g+4q+i)*128+p, core*2048 + 2w + b]
    A = Vb.reshape(NSB, 2, 4, 128, NCORES, NPC // 2, 2)
    words = np.zeros((NSB, 2, 128, NCORES, NPC // 2), np.uint16)
    for i in range(4):
        for b in range(2):
            words |= A[:, :, i, :, :, :, b].astype(np.uint16) << (i + 8 * b)
    # -> (core, g, p, q, w), packed superblocks only, in processing order
    words = np.ascontiguousarray(
        words[list(PACKED_SBS)].transpose(3, 0, 2, 1, 4))

    # dense slab (SBs 12,13): byte 0x01 per set vote, plane = cl*4 + pg,
    # plane (cl, pg) holds chunk DENSE_BASE + 4*pg + cl
    Vd = Vb.reshape(KC, 128, NCORES, NPC)[
        DENSE_BASE // CPS * CPS:DENSE_BASE + PPS]  # chunks 96..111
    dense = np.zeros((NCORES, 128, 4, 4, NPC), np.uint8)
    for cl in range(4):
        for pg in range(4):
            c = 4 * pg + cl
            dense[:, :, cl, pg, :] = np.ascontiguousarray(
                Vd[c].transpose(1, 0, 2)).astype(np.uint8)
    dense = dense.reshape(NCORES, 128, PPS * NPC).view(f8np)

    in_maps = []
    for n in range(NCORES):
        in_maps.append({"x": x_arr,
                        "v": words[n].reshape(NPK, 128, NPC),
                        "vd": dense[n]})
    return in_maps


def _run(image, vote_index, **run_kwargs):
    nc = _build()
    in_maps = _prep_inputs(np.asarray(image), np.asarray(vote_index))
    res = run_bass_kernel_spmd(nc, in_maps, core_ids=list(range(NCORES)),
                               **run_kwargs)
    # per-core out is (bin_lo 128, ng, m) -> (m, ng*128+bin_lo)
    cores = [r["out"].reshape(128, NG, BC).transpose(2, 1, 0)
             .reshape(BC, NPC) for r in res.results]
    out = np.concatenate(cores, axis=1)
    return out.reshape(B, C, H, W).astype(np.float32), res


def kernel(image: np.ndarray, vote_index: np.ndarray) -> np.ndarray:
    out, _ = _run(image, vote_index)
    return out


# revision 30
# speedup vs baseline: 1.0359x; 1.0359x over previous
"""Trainium2 Bass kernel for nn_HT_56298431316042 (histogram_binning).

Computes  out = relu(image.reshape(32, 16384)) @ vote.reshape(16384, 16384) / 128
         -> reshape (2, 16, 128, 128)

Sharding: column-wise over the 16384 Hough bins -> 2048 bins per core, 8 cores,
no communication.

The vote matrix is binary, so 1 byte/entry (fp8) wastes 7 bits. This kernel
ships most of V bit-packed at 4 entries/byte and expands on-device:

  - Host packs V[k, j] into uint16 words: word (p, g, q, w) carries, at bit
    i the entry for k-chunk (8g+4q+i) / bin 2w, and at bit (8+i) the same
    chunk / bin 2w+1.  (k-chunk = 128 consecutive k rows; p = k % 128.)
  - DVE tensor_scalar (w >> i) & 0x0101 (one dual-op instruction per
    bit-class, uint16 4x perf mode) extracts one byte per bin whose bit
    pattern is 0x01: as fp8e4m3 that is exactly 2^-9, so every expanded
    plane equals V * 2^-9 elementwise.
  - x is pre-scaled on host by 2^9 * 2^-7 = 4 (the 2^-7 is the reference's
    /128), so psum accumulates the final output directly; fp8 quantization
    of relu(x*4) is the only rounding (rel err ~3.7e-3).
  - The expanded planes are the matmul's *stationary* operand and x (just
    32 wide) the moving one, with psum indexed by bin; HBM traffic for V
    drops 4x (33.6 -> 8.4 MB/core) and the DVE expansion (~33 us) is the
    pipeline spine.
  - Two superblocks ride as pre-expanded fp8 planes straight from HBM
    (byte 0x01 = same 2^-9 value), trading idle DMA bandwidth for DVE time
    to balance the two spines.

Engine budget per core (cost model): DVE expand ~33 us / DMA ~34 us
(balanced spines), PE ~9 us, ACT only x-relu + epilogue staging.  V DMAs
issue from the SP queue so descriptor generation never stalls the ACT-side
x prep; x ships in two chunks so the first matmuls start ~5 us in.
"""

import numpy as np

import concourse.bass as bass
import concourse.bacc as bacc
import concourse.mybir as mybir
import concourse.tile as tile
from concourse.bass_utils import run_bass_kernel_spmd

NCORES = 8
B, C, ROWS, COLS, H, W = 2, 16, 128, 128, 128, 128
BC = B * C                      # 32 output rows
K = ROWS * COLS                 # 16384 contraction
NTOT = H * W                    # 16384 output bins
NPC = NTOT // NCORES            # 2048 bins per core
KC = K // 128                   # 128 k-chunks of 128 rows
NSB = 16                        # superblocks of 8 k-chunks
CPS = KC // NSB                 # 8 chunks per superblock
PPS = 2 * CPS                   # planes (chunks) per 2-SB slab = 16
NG = NPC // 128                 # psum bin groups
RSLAB = 3                       # expanded-ring depth (slabs)
XCH0 = 32                       # chunks covered by the first x chunk

# packed superblocks, in processing order; SBs 12,13 ship dense
PACKED_SBS = (0, 1, 2, 3, 4, 5, 6, 7, 8, 9, 10, 11, 14, 15)
NPK = len(PACKED_SBS)
DENSE_BASE = 96                 # first chunk of the dense slab (SB 12)

# plane byte 0x01 = e4m3 2^-9; x carries 2^9 * 2^-7 (the /128) = 4.0
X_SCALE = 4.0

_nc_cache: dict[str, bass.Bass] = {}


def _build() -> bass.Bass:
    if "bp4" in _nc_cache:
        return _nc_cache["bp4"]
    f32 = mybir.dt.float32
    f8 = mybir.dt.float8e4
    u16 = mybir.dt.uint16

    nc = bacc.Bacc("TRN2", target_bir_lowering=False, debug=False,
                   num_devices=NCORES)
    x_dram = nc.dram_tensor("x", (128, KC * BC), f8, kind="ExternalInput")
    v_dram = nc.dram_tensor("v", (NPK, 128, NPC), u16, kind="ExternalInput")
    d_dram = nc.dram_tensor("vd", (128, PPS * NPC), f8, kind="ExternalInput")
    o_dram = nc.dram_tensor("out", (128, NG * BC), f32, kind="ExternalOutput")

    with tile.TileContext(nc) as tc:
        with tc.tile_pool(name="xp", bufs=1) as xp, \
             tc.tile_pool(name="vp", bufs=1) as vp, \
             tc.tile_pool(name="sp", bufs=1) as sp, \
             tc.tile_pool(name="dp", bufs=1) as dp, \
             tc.tile_pool(name="pp", bufs=1, space="PSUM") as pp, \
             tc.tile_pool(name="op", bufs=1) as op:

            x_raw = xp.tile([128, KC * BC], f8)
            packed = vp.tile([128, NPK * NPC], u16)
            ring = sp.tile([128, RSLAB * PPS * NPC], f8)
            dense = dp.tile([128, PPS * NPC], f8)
            x8 = xp.tile([128, KC * BC], f8)
            psum = pp.tile([128, NG * BC], f32)

            # DMA-engine arrival order: x chunk 0, vq0, vq1, x chunk 1,
            # vq2..vq13, dense slab, output.  (x head on the ACT queue, all
            # V + x tail on the SP queue so descriptor generation never
            # stalls the ACT-side relu chain.)
            nc.scalar.dma_start(out=x_raw[:, :XCH0 * BC],
                                in_=x_dram.ap()[:, :XCH0 * BC])
            nc.sync.dma_start(out=packed[:, 0 * NPC:1 * NPC],
                              in_=v_dram.ap()[0])
            nc.sync.dma_start(out=packed[:, 1 * NPC:2 * NPC],
                              in_=v_dram.ap()[1])
            nc.sync.dma_start(out=packed[:, 2 * NPC:3 * NPC],
                              in_=v_dram.ap()[2])
            nc.sync.dma_start(out=packed[:, 3 * NPC:4 * NPC],
                              in_=v_dram.ap()[3])
            nc.sync.dma_start(out=x_raw[:, XCH0 * BC:],
                              in_=x_dram.ap()[:, XCH0 * BC:])
            for g in range(4, NPK):
                nc.sync.dma_start(out=packed[:, g * NPC:(g + 1) * NPC],
                                  in_=v_dram.ap()[g])
            HD = (PPS // 2) * NPC
            nc.sync.dma_start(out=dense[:, :HD], in_=d_dram.ap()[:, :HD])
            nc.sync.dma_start(out=dense[:, HD:], in_=d_dram.ap()[:, HD:])

            relu = mybir.ActivationFunctionType.Relu
            nc.scalar.activation(x8[:, :XCH0 * BC], x_raw[:, :XCH0 * BC], relu)
            nc.scalar.activation(x8[:, XCH0 * BC:], x_raw[:, XCH0 * BC:], relu)

            DR = mybir.MatmulPerfMode.DoubleRow
            shr = mybir.AluOpType.logical_shift_right
            band = mybir.AluOpType.bitwise_and
            first = [True]

            def mms(chunk_base, view, jj, pgs):
                """MMs for DR pairs (classes 2jj, 2jj+1), planes cl*4+pg."""
                for pg in pgs:
                    c = chunk_base + 4 * pg + 2 * jj
                    xmov = x8[:, c * BC:(c + 2) * BC].rearrange(
                        "p (j m) -> p j m", j=2)
                    for ng in range(NG):
                        nc.tensor.matmul(
                            psum[:, ng * BC:(ng + 1) * BC],
                            lhsT=view[:, 2 * jj:2 * jj + 2, pg,
                                      ng * 128:(ng + 1) * 128],
                            rhs=xmov,
                            start=first[0], stop=(c == KC - 2),
                            perf_mode=DR)
                        first[0] = False

            def and_op(out_ap, pin, i):
                nc.vector.tensor_scalar(
                    out=out_ap, in0=pin, scalar1=i, scalar2=0x0101,
                    op0=shr, op1=band)

            # slot schedule: (ring slab, packed tiles, plane cols, chunks)
            # slot 0 is split into two 1-SB halves for an earlier start
            dense8 = dense[:].rearrange("p (cl pg n) -> p cl pg n",
                                        cl=4, pg=4)
            n2slots = NPK // 2 - 1          # 2-SB slots after the split
            for s in range(1 + n2slots):
                slab = s % RSLAB
                if s <= 1:
                    # 1-SB half-slots: each AND group needs only one packed
                    # DMA, so the DVE stream starts ~3 us earlier
                    sbase = 2 * s * CPS
                    roff = slab * PPS * NPC
                    rslot = ring[:, roff:roff + PPS * NPC]
                    r16 = rslot.bitcast(u16).rearrange(
                        "p (cl pg w) -> p cl pg w", cl=4, pg=4)
                    r8 = rslot.rearrange("p (cl pg n) -> p cl pg n",
                                         cl=4, pg=4)
                    for h in range(2):
                        pin = packed[:, (2 * s + h) * NPC:
                                     (2 * s + h + 1) * NPC]
                        for i in (0, 1):
                            and_op(r16[:, i, 2 * h:2 * h + 2], pin, i)
                    mms(sbase, r8, 0, range(4))
                    for h in range(2):
                        pin = packed[:, (2 * s + h) * NPC:
                                     (2 * s + h + 1) * NPC]
                        for i in (2, 3):
                            and_op(r16[:, i, 2 * h:2 * h + 2], pin, i)
                    mms(sbase, r8, 1, range(4))
                else:
                    pk = (2 * s, 2 * s + 1)
                    chunk_base = PACKED_SBS[pk[0]] * CPS
                    roff = slab * PPS * NPC
                    rslot = ring[:, roff:roff + PPS * NPC]
                    r16 = rslot.bitcast(u16).rearrange(
                        "p (cl w) -> p cl w", cl=4)
                    r8 = rslot.rearrange("p (cl pg n) -> p cl pg n",
                                         cl=4, pg=4)
                    pin = packed[:, pk[0] * NPC:(pk[0] + 2) * NPC]
                    for i in (0, 1):
                        and_op(r16[:, i], pin, i)
                    mms(chunk_base, r8, 0, range(4))
                    for i in (2, 3):
                        and_op(r16[:, i], pin, i)
                    mms(chunk_base, r8, 1, range(4))
                if s == 4:
                    # dense slab MMs, first half (classes 0,1); the DMA
                    # lands late but the PE has plenty of slack here
                    mms(DENSE_BASE, dense8, 0, range(4))
                if s == n2slots - 1:
                    # second dense half just before the final packed slot,
                    # so only ~32 matmuls trail the last AND
                    mms(DENSE_BASE, dense8, 1, range(4))

            # psum already holds out (the /128 is folded into x); PSUM can't
            # feed DMA directly, so stage halves via DVE and ACT in parallel
            NF = NG * BC
            out_t = op.tile([128, NF], f32)
            HN = NF // 2
            nc.vector.tensor_copy(out=out_t[:, :HN], in_=psum[:, :HN])
            nc.scalar.copy(out_t[:, HN:], psum[:, HN:])
            nc.sync.dma_start(out=o_dram.ap()[:, :HN], in_=out_t[:, :HN])
            nc.scalar.dma_start(out=o_dram.ap()[:, HN:], in_=out_t[:, HN:])

    nc.finalize()
    _nc_cache["bp4"] = nc
    return nc


def _prep_inputs(image: np.ndarray, vote_index: np.ndarray):
    f8np = mybir.dt.np(mybir.dt.float8e4)

    # x arranged (128, c*32+m) = image[m, c*128+p] * 4, signed fp8
    x2 = np.ascontiguousarray(image.reshape(BC, K), dtype=np.float32)
    x3 = x2.reshape(BC, KC, 128) * X_SCALE
    x_arr = np.ascontiguousarray(
        x3.transpose(2, 1, 0)).reshape(128, KC * BC).astype(f8np)

    Vb = (vote_index.reshape(K, NTOT) != 0)
    # packed V: word[g, p, q*1024+w] bits (i | 8+i) =
    #   V[(8g+4q+i)*128+p, core*2048 + 2w + b]
    A = Vb.reshape(NSB, 2, 4, 128, NCORES, NPC // 2, 2)
    words = np.zeros((NSB, 2, 128, NCORES, NPC // 2), np.uint16)
    for i in range(4):
        for b in range(2):
            words |= A[:, :, i, :, :, :, b].astype(np.uint16) << (i + 8 * b)
    # -> (core, g, p, q, w), packed superblocks only, in processing order
    words = np.ascontiguousarray(
        words[list(PACKED_SBS)].transpose(3, 0, 2, 1, 4))

    # dense slab (SBs 12,13): byte 0x01 per set vote, plane = cl*4 + pg,
    # plane (cl, pg) holds chunk DENSE_BASE + 4*pg + cl
    Vd = Vb.reshape(KC, 128, NCORES, NPC)[
        DENSE_BASE // CPS * CPS:DENSE_BASE + PPS]  # chunks 96..111
    dense = np.zeros((NCORES, 128, 4, 4, NPC), np.uint8)
    for cl in range(4):
        for pg in range(4):
            c = 4 * pg + cl
            dense[:, :, cl, pg, :] = np.ascontiguousarray(
                Vd[c].transpose(1, 0, 2)).astype(np.uint8)
    dense = dense.reshape(NCORES, 128, PPS * NPC).view(f8np)

    in_maps = []
    for n in range(NCORES):
        in_maps.append({"x": x_arr,
                        "v": words[n].reshape(NPK, 128, NPC),
                        "vd": dense[n]})
    return in_maps


def _run(image, vote_index, **run_kwargs):
    nc = _build()
    in_maps = _prep_inputs(np.asarray(image), np.asarray(vote_index))
    res = run_bass_kernel_spmd(nc, in_maps, core_ids=list(range(NCORES)),
                               **run_kwargs)
    # per-core out is (bin_lo 128, ng, m) -> (m, ng*128+bin_lo)
    cores = [r["out"].reshape(128, NG, BC).transpose(2, 1, 0)
             .reshape(BC, NPC) for r in res.results]
    out = np.concatenate(cores, axis=1)
    return out.reshape(B, C, H, W).astype(np.float32), res


def kernel(image: np.ndarray, vote_index: np.ndarray) -> np.ndarray:
    out, _ = _run(image, vote_index)
    return out
